# revision 33
# baseline (speedup 1.0000x reference)
"""CenterLoss (segment_reduce) Trainium2 Bass kernel.

Distribution strategy: the centers table [C, E] is sharded row-wise across the
8 NeuronCores (C/8 rows each).  On the host, (embedding, label) pairs are
routed to the core owning the label's shard (sort by label == the all-to-all
route), along with small routing metadata (per-row slot ids for the sorted
unique labels, per-slot table indices and inverse counts).

Each core then:
  1. bulk-copies its centers shard to the output table (DRAM->DRAM),
  2. computes per-row norms, normalized rows x/||x|| and scaled rows x*||x||,
  3. computes segment sums over slots (unique labels) with one-hot matmuls
     accumulating into PSUM, 128 slots (one PSUM partition group) at a time,
  4. gathers the touched center rows via indirect DMA, applies the EMA update
     + renormalization, and scatters the updated rows over the bulk copy,
  5. computes its partial MSE loss via the expansion
     sum ||x||^2 - 2*sum_slots c.S2 + sum_slots q*||c||^2   (no per-row gather).

The host sums the 8 partial losses and concatenates the 8 output shards.
"""

import numpy as np

import concourse.bass as bass
import concourse.bacc as bacc
import concourse.mybir as mybir
import concourse.tile as tile
from concourse.bass_utils import run_bass_kernel_spmd

N_CORES = 8
P = 128
EMB = 256
F32 = mybir.dt.float32
I32 = mybir.dt.int32

_prog_cache: dict = {}


def _ceil_div(a, b):
    return -(-a // b)


def route(embeddings, centers, labels):
    """Host-side routing: shard centers row-wise, route sorted (emb, label)
    pairs to the owning core, and build per-core packed device inputs."""
    emb = np.ascontiguousarray(np.asarray(embeddings, dtype=np.float32))
    cen = np.ascontiguousarray(np.asarray(centers, dtype=np.float32))
    lab = np.asarray(labels).astype(np.int64)
    B = lab.shape[0]
    C = cen.shape[0]
    assert C % N_CORES == 0, f"centers rows {C} not divisible by {N_CORES}"
    shard = C // N_CORES

    order = np.argsort(lab, kind="stable")
    sl = lab[order]
    bounds = np.searchsorted(sl, np.arange(0, C + shard, shard))
    counts = np.diff(bounds)
    T = max(1, _ceil_div(int(counts.max()), P))
    B_CAP = T * P

    infos = []
    u_max = 1
    for c in range(N_CORES):
        seg = order[bounds[c]:bounds[c + 1]]
        ll = sl[bounds[c]:bounds[c + 1]] - c * shard
        if len(ll):
            uniq, inv, cnt = np.unique(ll, return_inverse=True, return_counts=True)
        else:
            uniq = np.zeros((0,), np.int64)
            inv = np.zeros((0,), np.int64)
            cnt = np.zeros((0,), np.int64)
        infos.append((seg, uniq, inv, cnt))
        u_max = max(u_max, len(uniq))
    G = max(1, _ceil_div(u_max, P))
    U_CAP = G * P

    pairs = set()
    per_core = []
    for c in range(N_CORES):
        seg, uniq, inv, cnt = infos[c]
        n = len(seg)
        u = len(uniq)

        e = np.zeros((B_CAP, EMB), np.float32)
        e[:n] = emb[seg]
        # pack [B_CAP, EMB] -> [P, T*EMB] so row t*P+p lands on partition p
        embp = np.ascontiguousarray(
            e.reshape(T, P, EMB).transpose(1, 0, 2).reshape(P, T * EMB))

        slot = np.full((B_CAP,), U_CAP - 1, np.int64)
        slot[:n] = inv
        slotf = np.ascontiguousarray(slot.reshape(T, P).T.astype(np.float32))

        gidx = np.zeros((U_CAP,), np.int32)
        gidx[:u] = uniq.astype(np.int32)
        # pad slots scatter into the trash row (index `shard`) of the output
        sidx = np.full((U_CAP,), shard, np.int32)
        sidx[:u] = uniq.astype(np.int32)
        invc = np.zeros((U_CAP,), np.float32)
        invc[:u] = (1.0 / cnt).astype(np.float32)

        per_core.append({
            "centers": np.ascontiguousarray(cen[c * shard:(c + 1) * shard]),
            "embp": embp,
            "slotf": slotf,
            "gidx": np.ascontiguousarray(gidx.reshape(G, P).T),
            "sidx": np.ascontiguousarray(sidx.reshape(G, P).T),
            "invc": np.ascontiguousarray(invc.reshape(G, P).T),
        })
        for t in range(T):
            glo = int(slot[t * P]) // P
            ghi = int(slot[t * P + P - 1]) // P
            for g in range(glo, ghi + 1):
                pairs.add((t, g))

    tiles_of = []
    for g in range(G):
        tl = sorted(t for (t, gg) in pairs if gg == g)
        tiles_of.append(tuple(tl) if tl else (0,))

    # For each slot group g, the table-row band it scatters into (union over
    # cores).  Slots are sorted unique labels, so group g covers a contiguous
    # row range per core; the union lets the shared program order scatter g
    # after only the copy chunks overlapping that band.
    bands = []
    for g in range(G):
        lo, hi = shard, -1
        for c in range(N_CORES):
            uniq = infos[c][1]
            u = len(uniq)
            s0, s1 = g * P, min((g + 1) * P, u) - 1
            if s0 >= u:
                continue
            lo = min(lo, int(uniq[s0]))
            hi = max(hi, int(uniq[s1]))
        bands.append((lo, hi))  # empty group: (shard, -1)
    return per_core, shard, B, T, G, tuple(tiles_of), tuple(bands)


FEATURES_ALL = frozenset({"copy", "rows", "groups", "scatter"})
COPY_NCH = 16
COPY_MODE = "d2d"  # "d2d" | "bounce"


def build_program(shard, T, G, tiles_of, bands=None, loop_reps=1,
                  timing_mode=False, features=FEATURES_ALL):
    """Build + compile the per-core Bass program (shared by all 8 cores).

    timing_mode=True turns out_centers into internal DRAM scratch (same
    memory traffic, tiny external outputs) so repeated dispatches are cheap.
    """
    nc = bacc.Bacc("TRN2", target_bir_lowering=False, debug=False,
                   num_devices=N_CORES)
    AF = mybir.ActivationFunctionType
    OP = mybir.AluOpType
    AX = mybir.AxisListType

    centers = nc.dram_tensor("centers", [shard, EMB], F32, kind="ExternalInput")
    embp = nc.dram_tensor("embp", [P, T * EMB], F32, kind="ExternalInput")
    slotf = nc.dram_tensor("slotf", [P, T], F32, kind="ExternalInput")
    gidx = nc.dram_tensor("gidx", [P, G], I32, kind="ExternalInput")
    sidx = nc.dram_tensor("sidx", [P, G], I32, kind="ExternalInput")
    invc = nc.dram_tensor("invc", [P, G], F32, kind="ExternalInput")
    # one extra trash row at index `shard` catches pad-slot scatters
    if timing_mode:
        out_c = nc.dram_tensor("out_centers_scratch", [shard + 1, EMB], F32)
        # tiny observable output reading back one row per copy chunk, so the
        # table writes cannot be elided / left un-awaited in the timing build
        out_probe = nc.dram_tensor("probe", [P, EMB], F32,
                                   kind="ExternalOutput")
    else:
        out_c = nc.dram_tensor("out_centers", [shard + 1, EMB], F32,
                               kind="ExternalOutput")
        out_probe = None
    out_l = nc.dram_tensor("loss_part", [P, 1], F32, kind="ExternalOutput")

    with tile.TileContext(nc) as tc:
        with (
            tc.tile_pool(name="const", bufs=1) as constp,
            tc.tile_pool(name="big", bufs=1) as bigp,
            tc.tile_pool(name="work", bufs=3) as workp,
            tc.tile_pool(name="grp", bufs=3) as grpp,
            tc.tile_pool(name="ps", bufs=2, space="PSUM") as psp,
        ):
            iota_i = constp.tile([P, P], I32)
            nc.gpsimd.iota(iota_i[:], pattern=[[1, P]], base=0,
                           channel_multiplier=0)
            iota_f = constp.tile([P, P], F32)
            nc.vector.tensor_copy(iota_f[:], iota_i[:])
            slotf_t = constp.tile([P, T], F32)
            nc.sync.dma_start(out=slotf_t[:], in_=slotf.ap())
            gidx_t = constp.tile([P, G], I32)
            nc.sync.dma_start(out=gidx_t[:], in_=gidx.ap())
            sidx_t = constp.tile([P, G], I32)
            nc.sync.dma_start(out=sidx_t[:], in_=sidx.ap())
            invc_t = constp.tile([P, G], F32)
            nc.sync.dma_start(out=invc_t[:], in_=invc.ap())

            def body(_iv=None):
                # embeddings load first: it heads the sync HWDGE FIFO and
                # unblocks all compute while the bulk copy streams
                X = None
                if "rows" in features:
                    X = bigp.tile([P, T * EMB], F32, tag="X")
                    nc.sync.dma_start(out=X[:], in_=embp.ap())

                # 1) bulk copy of the centers shard into the output table,
                # split across both HWDGE rings (sync + scalar).  Chunk k
                # covers rows [row_lo_k, row_hi_k) so scatters can order
                # against only the chunks they overlap.
                copy_raws = []       # (row_lo, row_hi, raw_inst)
                if "copy" in features:
                    flat_i = centers.ap().rearrange("a b -> (a b)")
                    flat_o = out_c.ap().rearrange("a b -> (a b)")
                    nch = COPY_NCH
                    rows_per = _ceil_div(shard, nch)
                    for k in range(nch):
                        rlo = k * rows_per
                        rhi = min((k + 1) * rows_per, shard)
                        if rlo >= rhi:
                            continue
                        eng = nc.sync if k % 2 == 0 else nc.scalar
                        if COPY_MODE == "bounce":
                            w = (rhi - rlo) * EMB // P
                            cpt = workp.tile([P, w], F32, tag="cpt")
                            eng.dma_start(out=cpt[:],
                                          in_=flat_i[rlo * EMB:rhi * EMB])
                            eng.dma_start(out=flat_o[rlo * EMB:rhi * EMB],
                                          in_=cpt[:])
                        else:
                            eng.dma_start(out=flat_o[rlo * EMB:rhi * EMB],
                                          in_=flat_i[rlo * EMB:rhi * EMB])
                        copy_raws.append((rlo, rhi, nc.cur_bb.bb.instructions[-1]))

                def finish(lacc_tile):
                    nc.sync.dma_start(out=out_l.ap(), in_=lacc_tile[:])
                    if timing_mode and out_probe is not None and copy_raws:
                        # one strided read touching every copy chunk's range
                        n = len(copy_raws)
                        stride = max(1, shard // P)
                        prb = workp.tile([P, EMB], F32, tag="prb")
                        nc.sync.dma_start(
                            out=prb[:],
                            in_=out_c.ap()[:stride * P:stride, :])
                        nc.sync.dma_start(out=out_probe.ap(), in_=prb[:])

                lacc = workp.tile([P, 1], F32, tag="lacc")
                if "rows" not in features:
                    nc.vector.memset(lacc[:], 0.0)
                    finish(lacc)
                    return
                Xc = bigp.tile([P, T * 2 * EMB], F32, tag="Xc")
                ss_all = bigp.tile([P, T], F32, tag="ss")
                nrm_all = bigp.tile([P, T], F32, tag="nrm")
                inv_all = bigp.tile([P, T], F32, tag="inv")
                for t in range(T):
                    x = X[:, t * EMB:(t + 1) * EMB]
                    sq = workp.tile([P, EMB], F32, tag="sq")
                    nc.scalar.activation(out=sq[:], in_=x, func=AF.Square,
                                         accum_out=ss_all[:, t:t + 1])
                    nc.scalar.sqrt(nrm_all[:, t:t + 1], ss_all[:, t:t + 1])
                    gc = workp.tile([P, 1], F32, tag="gc")
                    nc.vector.tensor_scalar_max(gc[:], nrm_all[:, t:t + 1], 1e-30)
                    nc.vector.reciprocal(inv_all[:, t:t + 1], gc[:])
                    nc.scalar.activation(out=Xc[:, (2 * t) * EMB:(2 * t + 1) * EMB],
                                         in_=x, func=AF.Copy,
                                         scale=inv_all[:, t:t + 1])
                    nc.scalar.activation(out=Xc[:, (2 * t + 1) * EMB:(2 * t + 2) * EMB],
                                         in_=x, func=AF.Copy,
                                         scale=nrm_all[:, t:t + 1])

                nc.vector.tensor_reduce(out=lacc[:], in_=ss_all[:], axis=AX.X,
                                        op=OP.add)

                if "groups" not in features:
                    finish(lacc)
                    return

                # 3) per slot-group: segment sums -> EMA update.  Updated
                # rows land in a resident buffer; all scatters are issued
                # afterwards so a scatter stalled on the bulk copy never
                # head-of-line-blocks the gathers on the Pool engine.
                orow_all = bigp.tile([P, G * EMB], F32, tag="orow_all")
                for g in range(G):
                    tl = tiles_of[g]
                    psA = psp.tile([P, 2 * EMB], F32, tag="psA")
                    psQ = psp.tile([P, 1], F32, tag="psQ")
                    for j, t in enumerate(tl):
                        oh = workp.tile([P, P], F32, tag="oh")
                        # oh[i, m] = 1.0 iff slot(i) == g*128 + m
                        nc.vector.tensor_scalar(
                            out=oh[:], in0=iota_f[:],
                            scalar1=slotf_t[:, t:t + 1], scalar2=float(-g * P),
                            op0=OP.subtract, op1=OP.is_equal)
                        st = (j == 0)
                        sp = (j == len(tl) - 1)
                        nc.tensor.matmul(out=psA[:, :], lhsT=oh[:],
                                         rhs=Xc[:, (2 * t) * EMB:(2 * t + 2) * EMB],
                                         start=st, stop=sp)
                        nc.tensor.matmul(out=psQ[:, :], lhsT=oh[:],
                                         rhs=ss_all[:, t:t + 1],
                                         start=st, stop=sp)

                    cg = grpp.tile([P, EMB], F32, tag="cg")
                    nc.gpsimd.indirect_dma_start(
                        out=cg[:], out_offset=None, in_=centers.ap(),
                        in_offset=bass.IndirectOffsetOnAxis(
                            ap=gidx_t[:, g:g + 1], axis=0))
                    mean = grpp.tile([P, EMB], F32, tag="mean")
                    nc.vector.tensor_scalar(out=mean[:], in0=psA[:, 0:EMB],
                                            scalar1=invc_t[:, g:g + 1],
                                            scalar2=None, op0=OP.mult)
                    upd = grpp.tile([P, EMB], F32, tag="upd")
                    nc.vector.tensor_tensor(out=upd[:], in0=cg[:], in1=mean[:],
                                            op=OP.add)
                    squ = grpp.tile([P, EMB], F32, tag="squ")
                    ssu = grpp.tile([P, 1], F32, tag="ssu")
                    nc.scalar.activation(out=squ[:], in_=upd[:], func=AF.Square,
                                         accum_out=ssu[:])
                    nu = grpp.tile([P, 1], F32, tag="nu")
                    # ||0.5*(c+mean)|| = sqrt(0.25 * sum((c+mean)^2))
                    nc.scalar.activation(out=nu[:], in_=ssu[:], func=AF.Sqrt,
                                         scale=0.25)
                    # 2*max(nu, 1e-12); reciprocal gives 0.5/max(nu, 1e-12)
                    nc.vector.tensor_scalar(out=nu[:], in0=nu[:], scalar1=1e-12,
                                            scalar2=2.0, op0=OP.max, op1=OP.mult)
                    rcol = grpp.tile([P, 1], F32, tag="rcol")
                    nc.vector.reciprocal(rcol[:], nu[:])
                    nc.scalar.activation(out=orow_all[:, g * EMB:(g + 1) * EMB],
                                         in_=upd[:], func=AF.Copy,
                                         scale=rcol[:])

                    # loss terms: -2 * c.S2 and q * ||c||^2
                    l1o = grpp.tile([P, EMB], F32, tag="l1o")
                    t1 = grpp.tile([P, 1], F32, tag="t1")
                    nc.vector.tensor_tensor(out=l1o[:], in0=cg[:],
                                            in1=psA[:, EMB:2 * EMB], op=OP.mult)
                    nc.vector.tensor_reduce(out=t1[:], in_=l1o[:], axis=AX.X,
                                            op=OP.add)
                    nc.vector.tensor_scalar(out=t1[:], in0=t1[:], scalar1=-2.0,
                                            scalar2=None, op0=OP.mult)
                    l2o = grpp.tile([P, EMB], F32, tag="l2o")
                    t2 = grpp.tile([P, 1], F32, tag="t2")
                    nc.scalar.activation(out=l2o[:], in_=cg[:], func=AF.Square,
                                         accum_out=t2[:])
                    nc.vector.tensor_tensor(out=t2[:], in0=t2[:],
                                            in1=psQ[:, 0:1], op=OP.mult)
                    nc.vector.tensor_tensor(out=lacc[:], in0=lacc[:], in1=t1[:],
                                            op=OP.add)
                    nc.vector.tensor_tensor(out=lacc[:], in0=lacc[:], in1=t2[:],
                                            op=OP.add)

                # 4) scatter pass: all updated rows into the output table
                if "scatter" in features:
                    prev_scatter = None
                    for g in range(G):
                        nc.gpsimd.indirect_dma_start(
                            out=out_c.ap(),
                            out_offset=bass.IndirectOffsetOnAxis(
                                ap=sidx_t[:, g:g + 1], axis=0),
                            in_=orow_all[:, g * EMB:(g + 1) * EMB],
                            in_offset=None)
                        raw = nc.cur_bb.bb.instructions[-1]
                        # scatters write disjoint slot rows: drop the
                        # conservative scatter->scatter WAW chain, but keep
                        # the semantic scatter-after-bulk-copy ordering for
                        # the copy chunks this group's row band overlaps
                        if prev_scatter is not None:
                            raw.try_remove_dependency(prev_scatter.name)
                        if bands is not None:
                            blo, bhi = bands[g]
                        else:
                            blo, bhi = 0, shard - 1
                        for rlo, rhi, ci in copy_raws:
                            if rhi > blo and rlo <= bhi:
                                tile.add_dep_helper(
                                    raw, ci,
                                    reason="scatter lands after bulk copy")
                        prev_scatter = raw

                finish(lacc)

            if loop_reps == 1 and not timing_mode:
                body()
            else:
                with tc.For_i(0, loop_reps, 1) as _i:
                    body(_i)
                    # serialize iterations so the loop delta measures true
                    # single-shot latency, not pipelined throughput
                    tc.strict_bb_all_engine_barrier()

    nc.compile()
    return nc


def kernel(embeddings, centers, labels):
    per_core, shard, B, T, G, tiles_of, bands = route(
        embeddings, centers, labels)
    key = (shard, T, G, tiles_of, bands, 1, False)
    nc = _prog_cache.get(key)
    if nc is None:
        nc = build_program(shard, T, G, tiles_of, bands)
        _prog_cache[key] = nc
    res = run_bass_kernel_spmd(nc, per_core, core_ids=list(range(N_CORES)))
    new_centers = np.concatenate(
        [r["out_centers"][:shard] for r in res.results], axis=0)
    total = sum(float(r["loss_part"].astype(np.float64).sum())
                for r in res.results)
    loss = np.float32(total / (B * EMB))
    return loss, new_centers


# revision 38
# speedup vs baseline: 1.1365x; 1.1365x over previous
"""CenterLoss (segment_reduce) Trainium2 Bass kernel.

Distribution strategy: the centers table [C, E] is sharded row-wise across the
8 NeuronCores (C/8 rows each).  On the host, (embedding, label) pairs are
routed to the core owning the label's shard (sort by label == the all-to-all
route), along with small routing metadata (per-row slot ids for the sorted
unique labels, per-slot table indices and inverse counts).

Each core then:
  1. bulk-copies its centers shard to the output table (DRAM->DRAM),
  2. computes per-row norms, normalized rows x/||x|| and scaled rows x*||x||,
  3. computes segment sums over slots (unique labels) with one-hot matmuls
     accumulating into PSUM, 128 slots (one PSUM partition group) at a time,
  4. gathers the touched center rows via indirect DMA, applies the EMA update
     + renormalization, and scatters the updated rows over the bulk copy,
  5. computes its partial MSE loss via the expansion
     sum ||x||^2 - 2*sum_slots c.S2 + sum_slots q*||c||^2   (no per-row gather).

The host sums the 8 partial losses and concatenates the 8 output shards.
"""

import numpy as np

import concourse.bass as bass
import concourse.bacc as bacc
import concourse.mybir as mybir
import concourse.tile as tile
from concourse.bass_utils import run_bass_kernel_spmd

N_CORES = 8
P = 128
EMB = 256
F32 = mybir.dt.float32
I32 = mybir.dt.int32

_prog_cache: dict = {}


def _ceil_div(a, b):
    return -(-a // b)


def route(embeddings, centers, labels):
    """Host-side routing: shard centers row-wise, route sorted (emb, label)
    pairs to the owning core, and build per-core packed device inputs."""
    emb = np.ascontiguousarray(np.asarray(embeddings, dtype=np.float32))
    cen = np.ascontiguousarray(np.asarray(centers, dtype=np.float32))
    lab = np.asarray(labels).astype(np.int64)
    B = lab.shape[0]
    C = cen.shape[0]
    assert C % N_CORES == 0, f"centers rows {C} not divisible by {N_CORES}"
    shard = C // N_CORES

    order = np.argsort(lab, kind="stable")
    sl = lab[order]
    bounds = np.searchsorted(sl, np.arange(0, C + shard, shard))
    counts = np.diff(bounds)
    T = max(1, _ceil_div(int(counts.max()), P))
    B_CAP = T * P

    infos = []
    u_max = 1
    for c in range(N_CORES):
        seg = order[bounds[c]:bounds[c + 1]]
        ll = sl[bounds[c]:bounds[c + 1]] - c * shard
        if len(ll):
            uniq, inv, cnt = np.unique(ll, return_inverse=True, return_counts=True)
        else:
            uniq = np.zeros((0,), np.int64)
            inv = np.zeros((0,), np.int64)
            cnt = np.zeros((0,), np.int64)
        infos.append((seg, uniq, inv, cnt))
        u_max = max(u_max, len(uniq))
    G = max(1, _ceil_div(u_max, P))
    U_CAP = G * P

    pairs = set()
    per_core = []
    for c in range(N_CORES):
        seg, uniq, inv, cnt = infos[c]
        n = len(seg)
        u = len(uniq)

        e = np.zeros((B_CAP, EMB), np.float32)
        e[:n] = emb[seg]
        # pack [B_CAP, EMB] -> [P, T*EMB] so row t*P+p lands on partition p
        embp = np.ascontiguousarray(
            e.reshape(T, P, EMB).transpose(1, 0, 2).reshape(P, T * EMB))

        slot = np.full((B_CAP,), U_CAP - 1, np.int64)
        slot[:n] = inv
        slotf = np.ascontiguousarray(slot.reshape(T, P).T.astype(np.float32))

        gidx = np.zeros((U_CAP,), np.int32)
        gidx[:u] = uniq.astype(np.int32)
        # pad slots scatter into the trash row (index `shard`) of the output
        sidx = np.full((U_CAP,), shard, np.int32)
        sidx[:u] = uniq.astype(np.int32)
        invc = np.zeros((U_CAP,), np.float32)
        invc[:u] = (1.0 / cnt).astype(np.float32)

        per_core.append({
            "centers": np.ascontiguousarray(cen[c * shard:(c + 1) * shard]),
            "embp": embp,
            "slotf": slotf,
            "gidx": np.ascontiguousarray(gidx.reshape(G, P).T),
            "sidx": np.ascontiguousarray(sidx.reshape(G, P).T),
            "invc": np.ascontiguousarray(invc.reshape(G, P).T),
        })
        for t in range(T):
            glo = int(slot[t * P]) // P
            ghi = int(slot[t * P + P - 1]) // P
            for g in range(glo, ghi + 1):
                pairs.add((t, g))

    tiles_of = []
    for g in range(G):
        tl = sorted(t for (t, gg) in pairs if gg == g)
        tiles_of.append(tuple(tl) if tl else (0,))

    # For each slot group g, the table-row band it scatters into (union over
    # cores).  Slots are sorted unique labels, so group g covers a contiguous
    # row range per core; the union lets the shared program order scatter g
    # after only the copy chunks overlapping that band.
    bands = []
    for g in range(G):
        lo, hi = shard, -1
        for c in range(N_CORES):
            uniq = infos[c][1]
            u = len(uniq)
            s0, s1 = g * P, min((g + 1) * P, u) - 1
            if s0 >= u:
                continue
            lo = min(lo, int(uniq[s0]))
            hi = max(hi, int(uniq[s1]))
        bands.append((lo, hi))  # empty group: (shard, -1)
    return per_core, shard, B, T, G, tuple(tiles_of), tuple(bands)


FEATURES_ALL = frozenset({"copy", "rows", "groups", "scatter"})
COPY_NCH = 16
COPY_MODE = "d2d"  # "d2d" | "bounce"
SCATTER_INTERLEAVE = True  # issue scatter g right after group g's finalize
PSUM_BUFS = 2


def build_program(shard, T, G, tiles_of, bands=None, loop_reps=1,
                  timing_mode=False, features=FEATURES_ALL):
    """Build + compile the per-core Bass program (shared by all 8 cores).

    timing_mode=True turns out_centers into internal DRAM scratch (same
    memory traffic, tiny external outputs) so repeated dispatches are cheap.
    """
    nc = bacc.Bacc("TRN2", target_bir_lowering=False, debug=False,
                   num_devices=N_CORES)
    AF = mybir.ActivationFunctionType
    OP = mybir.AluOpType
    AX = mybir.AxisListType

    centers = nc.dram_tensor("centers", [shard, EMB], F32, kind="ExternalInput")
    embp = nc.dram_tensor("embp", [P, T * EMB], F32, kind="ExternalInput")
    slotf = nc.dram_tensor("slotf", [P, T], F32, kind="ExternalInput")
    gidx = nc.dram_tensor("gidx", [P, G], I32, kind="ExternalInput")
    sidx = nc.dram_tensor("sidx", [P, G], I32, kind="ExternalInput")
    invc = nc.dram_tensor("invc", [P, G], F32, kind="ExternalInput")
    # one extra trash row at index `shard` catches pad-slot scatters
    if timing_mode:
        out_c = nc.dram_tensor("out_centers_scratch", [shard + 1, EMB], F32)
        # tiny observable output reading back one row per copy chunk, so the
        # table writes cannot be elided / left un-awaited in the timing build
        out_probe = nc.dram_tensor("probe", [P, EMB], F32,
                                   kind="ExternalOutput")
    else:
        out_c = nc.dram_tensor("out_centers", [shard + 1, EMB], F32,
                               kind="ExternalOutput")
        out_probe = None
    out_l = nc.dram_tensor("loss_part", [P, 1], F32, kind="ExternalOutput")

    with tile.TileContext(nc) as tc:
        with (
            tc.tile_pool(name="const", bufs=1) as constp,
            tc.tile_pool(name="big", bufs=1) as bigp,
            tc.tile_pool(name="work", bufs=3) as workp,
            tc.tile_pool(name="grp", bufs=3) as grpp,
            tc.tile_pool(name="ps", bufs=PSUM_BUFS, space="PSUM") as psp,
        ):
            iota_i = constp.tile([P, P], I32)
            nc.gpsimd.iota(iota_i[:], pattern=[[1, P]], base=0,
                           channel_multiplier=0)
            iota_f = constp.tile([P, P], F32)
            nc.vector.tensor_copy(iota_f[:], iota_i[:])
            slotf_t = constp.tile([P, T], F32)
            nc.sync.dma_start(out=slotf_t[:], in_=slotf.ap())
            gidx_t = constp.tile([P, G], I32)
            nc.sync.dma_start(out=gidx_t[:], in_=gidx.ap())
            sidx_t = constp.tile([P, G], I32)
            nc.sync.dma_start(out=sidx_t[:], in_=sidx.ap())
            invc_t = constp.tile([P, G], F32)
            nc.sync.dma_start(out=invc_t[:], in_=invc.ap())

            def body(_iv=None):
                # embeddings load first: it heads the sync HWDGE FIFO and
                # unblocks all compute while the bulk copy streams
                X = None
                if "rows" in features:
                    X = bigp.tile([P, T * EMB], F32, tag="X")
                    nc.sync.dma_start(out=X[:], in_=embp.ap())

                # 1) bulk copy of the centers shard into the output table,
                # split across both HWDGE rings (sync + scalar).  Chunk k
                # covers rows [row_lo_k, row_hi_k) so scatters can order
                # against only the chunks they overlap.
                copy_raws = []       # (row_lo, row_hi, raw_inst)
                if "copy" in features:
                    flat_i = centers.ap().rearrange("a b -> (a b)")
                    flat_o = out_c.ap().rearrange("a b -> (a b)")
                    nch = COPY_NCH
                    rows_per = _ceil_div(shard, nch)
                    for k in range(nch):
                        rlo = k * rows_per
                        rhi = min((k + 1) * rows_per, shard)
                        if rlo >= rhi:
                            continue
                        eng = nc.sync if k % 2 == 0 else nc.scalar
                        if COPY_MODE == "bounce":
                            w = (rhi - rlo) * EMB // P
                            cpt = workp.tile([P, w], F32, tag="cpt")
                            eng.dma_start(out=cpt[:],
                                          in_=flat_i[rlo * EMB:rhi * EMB])
                            eng.dma_start(out=flat_o[rlo * EMB:rhi * EMB],
                                          in_=cpt[:])
                        else:
                            eng.dma_start(out=flat_o[rlo * EMB:rhi * EMB],
                                          in_=flat_i[rlo * EMB:rhi * EMB])
                        copy_raws.append((rlo, rhi, nc.cur_bb.bb.instructions[-1]))

                def finish(lacc_tile):
                    nc.sync.dma_start(out=out_l.ap(), in_=lacc_tile[:])
                    if timing_mode and out_probe is not None and copy_raws:
                        # one strided read touching every copy chunk's range
                        n = len(copy_raws)
                        stride = max(1, shard // P)
                        prb = workp.tile([P, EMB], F32, tag="prb")
                        nc.sync.dma_start(
                            out=prb[:],
                            in_=out_c.ap()[:stride * P:stride, :])
                        nc.sync.dma_start(out=out_probe.ap(), in_=prb[:])

                lacc = workp.tile([P, 1], F32, tag="lacc")
                if "rows" not in features:
                    nc.vector.memset(lacc[:], 0.0)
                    finish(lacc)
                    return
                Xc = bigp.tile([P, T * 2 * EMB], F32, tag="Xc")
                ss_all = bigp.tile([P, T], F32, tag="ss")
                nrm_all = bigp.tile([P, T], F32, tag="nrm")
                inv_all = bigp.tile([P, T], F32, tag="inv")
                for t in range(T):
                    x = X[:, t * EMB:(t + 1) * EMB]
                    sq = workp.tile([P, EMB], F32, tag="sq")
                    nc.scalar.activation(out=sq[:], in_=x, func=AF.Square,
                                         accum_out=ss_all[:, t:t + 1])
                    nc.scalar.sqrt(nrm_all[:, t:t + 1], ss_all[:, t:t + 1])
                    gc = workp.tile([P, 1], F32, tag="gc")
                    nc.vector.tensor_scalar_max(gc[:], nrm_all[:, t:t + 1], 1e-30)
                    nc.vector.reciprocal(inv_all[:, t:t + 1], gc[:])
                    nc.scalar.activation(out=Xc[:, (2 * t) * EMB:(2 * t + 1) * EMB],
                                         in_=x, func=AF.Copy,
                                         scale=inv_all[:, t:t + 1])
                    nc.scalar.activation(out=Xc[:, (2 * t + 1) * EMB:(2 * t + 2) * EMB],
                                         in_=x, func=AF.Copy,
                                         scale=nrm_all[:, t:t + 1])

                nc.vector.tensor_reduce(out=lacc[:], in_=ss_all[:], axis=AX.X,
                                        op=OP.add)

                if "groups" not in features:
                    finish(lacc)
                    return

                # 3) per slot-group: segment sums -> EMA update.  Updated
                # rows land in a resident buffer; all scatters are issued
                # afterwards so a scatter stalled on the bulk copy never
                # head-of-line-blocks the gathers on the Pool engine.
                orow_all = bigp.tile([P, G * EMB], F32, tag="orow_all")
                scatter_state = [None]

                def do_scatter(g):
                    nc.gpsimd.indirect_dma_start(
                        out=out_c.ap(),
                        out_offset=bass.IndirectOffsetOnAxis(
                            ap=sidx_t[:, g:g + 1], axis=0),
                        in_=orow_all[:, g * EMB:(g + 1) * EMB],
                        in_offset=None)
                    raw = nc.cur_bb.bb.instructions[-1]
                    # scatters write disjoint slot rows: drop the conservative
                    # scatter->scatter WAW chain, but keep the semantic
                    # scatter-after-bulk-copy ordering for the copy chunks
                    # this group's row band overlaps
                    if scatter_state[0] is not None:
                        raw.try_remove_dependency(scatter_state[0].name)
                    if bands is not None:
                        blo, bhi = bands[g]
                    else:
                        blo, bhi = 0, shard - 1
                    for rlo, rhi, ci in copy_raws:
                        if rhi > blo and rlo <= bhi:
                            tile.add_dep_helper(
                                raw, ci, reason="scatter lands after bulk copy")
                    scatter_state[0] = raw

                for g in range(G):
                    tl = tiles_of[g]
                    psA = psp.tile([P, 2 * EMB], F32, tag="psA")
                    psQ = psp.tile([P, 1], F32, tag="psQ")
                    for j, t in enumerate(tl):
                        oh = workp.tile([P, P], F32, tag="oh")
                        # oh[i, m] = 1.0 iff slot(i) == g*128 + m
                        nc.vector.tensor_scalar(
                            out=oh[:], in0=iota_f[:],
                            scalar1=slotf_t[:, t:t + 1], scalar2=float(-g * P),
                            op0=OP.subtract, op1=OP.is_equal)
                        st = (j == 0)
                        sp = (j == len(tl) - 1)
                        nc.tensor.matmul(out=psA[:, :], lhsT=oh[:],
                                         rhs=Xc[:, (2 * t) * EMB:(2 * t + 2) * EMB],
                                         start=st, stop=sp)
                        nc.tensor.matmul(out=psQ[:, :], lhsT=oh[:],
                                         rhs=ss_all[:, t:t + 1],
                                         start=st, stop=sp)

                    cg = grpp.tile([P, EMB], F32, tag="cg")
                    nc.gpsimd.indirect_dma_start(
                        out=cg[:], out_offset=None, in_=centers.ap(),
                        in_offset=bass.IndirectOffsetOnAxis(
                            ap=gidx_t[:, g:g + 1], axis=0))
                    mean = grpp.tile([P, EMB], F32, tag="mean")
                    nc.vector.tensor_scalar(out=mean[:], in0=psA[:, 0:EMB],
                                            scalar1=invc_t[:, g:g + 1],
                                            scalar2=None, op0=OP.mult)
                    upd = grpp.tile([P, EMB], F32, tag="upd")
                    nc.vector.tensor_tensor(out=upd[:], in0=cg[:], in1=mean[:],
                                            op=OP.add)
                    squ = grpp.tile([P, EMB], F32, tag="squ")
                    ssu = grpp.tile([P, 1], F32, tag="ssu")
                    nc.scalar.activation(out=squ[:], in_=upd[:], func=AF.Square,
                                         accum_out=ssu[:])
                    nu = grpp.tile([P, 1], F32, tag="nu")
                    # ||0.5*(c+mean)|| = sqrt(0.25 * sum((c+mean)^2))
                    nc.scalar.activation(out=nu[:], in_=ssu[:], func=AF.Sqrt,
                                         scale=0.25)
                    # 2*max(nu, 1e-12); reciprocal gives 0.5/max(nu, 1e-12)
                    nc.vector.tensor_scalar(out=nu[:], in0=nu[:], scalar1=1e-12,
                                            scalar2=2.0, op0=OP.max, op1=OP.mult)
                    rcol = grpp.tile([P, 1], F32, tag="rcol")
                    nc.vector.reciprocal(rcol[:], nu[:])
                    nc.scalar.activation(out=orow_all[:, g * EMB:(g + 1) * EMB],
                                         in_=upd[:], func=AF.Copy,
                                         scale=rcol[:])

                    # loss terms: -2 * c.S2 and q * ||c||^2
                    l1o = grpp.tile([P, EMB], F32, tag="l1o")
                    t1 = grpp.tile([P, 1], F32, tag="t1")
                    nc.vector.tensor_tensor(out=l1o[:], in0=cg[:],
                                            in1=psA[:, EMB:2 * EMB], op=OP.mult)
                    nc.vector.tensor_reduce(out=t1[:], in_=l1o[:], axis=AX.X,
                                            op=OP.add)
                    nc.vector.tensor_scalar(out=t1[:], in0=t1[:], scalar1=-2.0,
                                            scalar2=None, op0=OP.mult)
                    l2o = grpp.tile([P, EMB], F32, tag="l2o")
                    t2 = grpp.tile([P, 1], F32, tag="t2")
                    nc.scalar.activation(out=l2o[:], in_=cg[:], func=AF.Square,
                                         accum_out=t2[:])
                    nc.vector.tensor_tensor(out=t2[:], in0=t2[:],
                                            in1=psQ[:, 0:1], op=OP.mult)
                    nc.vector.tensor_tensor(out=lacc[:], in0=lacc[:], in1=t1[:],
                                            op=OP.add)
                    nc.vector.tensor_tensor(out=lacc[:], in0=lacc[:], in1=t2[:],
                                            op=OP.add)
                    if "scatter" in features and SCATTER_INTERLEAVE:
                        do_scatter(g)

                # 4) scatter pass: all updated rows into the output table
                if "scatter" in features and not SCATTER_INTERLEAVE:
                    for g in range(G):
                        do_scatter(g)

                finish(lacc)

            if loop_reps == 1 and not timing_mode:
                body()
            else:
                with tc.For_i(0, loop_reps, 1) as _i:
                    body(_i)
                    # serialize iterations so the loop delta measures true
                    # single-shot latency, not pipelined throughput
                    tc.strict_bb_all_engine_barrier()

    nc.compile()
    return nc


def kernel(embeddings, centers, labels):
    per_core, shard, B, T, G, tiles_of, bands = route(
        embeddings, centers, labels)
    key = (shard, T, G, tiles_of, bands, 1, False)
    nc = _prog_cache.get(key)
    if nc is None:
        nc = build_program(shard, T, G, tiles_of, bands)
        _prog_cache[key] = nc
    res = run_bass_kernel_spmd(nc, per_core, core_ids=list(range(N_CORES)))
    new_centers = np.concatenate(
        [r["out_centers"][:shard] for r in res.results], axis=0)
    total = sum(float(r["loss_part"].astype(np.float64).sum())
                for r in res.results)
    loss = np.float32(total / (B * EMB))
    return loss, new_centers


# revision 41
# speedup vs baseline: 1.2036x; 1.0591x over previous
"""CenterLoss (segment_reduce) Trainium2 Bass kernel.

Distribution strategy: the centers table [C, E] is sharded row-wise across the
8 NeuronCores (C/8 rows each).  On the host, (embedding, label) pairs are
routed to the core owning the label's shard (sort by label == the all-to-all
route), along with small routing metadata (per-row slot ids for the sorted
unique labels, per-slot table indices and inverse counts).

Each core then:
  1. bulk-copies its centers shard to the output table (DRAM->DRAM),
  2. computes per-row norms, normalized rows x/||x|| and scaled rows x*||x||,
  3. computes segment sums over slots (unique labels) with one-hot matmuls
     accumulating into PSUM, 128 slots (one PSUM partition group) at a time,
  4. gathers the touched center rows via indirect DMA, applies the EMA update
     + renormalization, and scatters the updated rows over the bulk copy,
  5. computes its partial MSE loss via the expansion
     sum ||x||^2 - 2*sum_slots c.S2 + sum_slots q*||c||^2   (no per-row gather).

The host sums the 8 partial losses and concatenates the 8 output shards.
"""

import numpy as np

import concourse.bass as bass
import concourse.bacc as bacc
import concourse.mybir as mybir
import concourse.tile as tile
from concourse.bass_utils import run_bass_kernel_spmd

N_CORES = 8
P = 128
EMB = 256
F32 = mybir.dt.float32
I32 = mybir.dt.int32

_prog_cache: dict = {}


def _ceil_div(a, b):
    return -(-a // b)


def route(embeddings, centers, labels):
    """Host-side routing: shard centers row-wise, route sorted (emb, label)
    pairs to the owning core, and build per-core packed device inputs."""
    emb = np.ascontiguousarray(np.asarray(embeddings, dtype=np.float32))
    cen = np.ascontiguousarray(np.asarray(centers, dtype=np.float32))
    lab = np.asarray(labels).astype(np.int64)
    B = lab.shape[0]
    C = cen.shape[0]
    assert C % N_CORES == 0, f"centers rows {C} not divisible by {N_CORES}"
    shard = C // N_CORES

    order = np.argsort(lab, kind="stable")
    sl = lab[order]
    bounds = np.searchsorted(sl, np.arange(0, C + shard, shard))
    counts = np.diff(bounds)
    T = max(1, _ceil_div(int(counts.max()), P))
    B_CAP = T * P

    infos = []
    u_max = 1
    for c in range(N_CORES):
        seg = order[bounds[c]:bounds[c + 1]]
        ll = sl[bounds[c]:bounds[c + 1]] - c * shard
        if len(ll):
            uniq, inv, cnt = np.unique(ll, return_inverse=True, return_counts=True)
        else:
            uniq = np.zeros((0,), np.int64)
            inv = np.zeros((0,), np.int64)
            cnt = np.zeros((0,), np.int64)
        infos.append((seg, uniq, inv, cnt))
        u_max = max(u_max, len(uniq))
    G = max(1, _ceil_div(u_max, P))
    U_CAP = G * P

    pairs = set()
    per_core = []
    for c in range(N_CORES):
        seg, uniq, inv, cnt = infos[c]
        n = len(seg)
        u = len(uniq)

        e = np.zeros((B_CAP, EMB), np.float32)
        e[:n] = emb[seg]
        # pack [B_CAP, EMB] -> [P, T*EMB] so row t*P+p lands on partition p
        embp = np.ascontiguousarray(
            e.reshape(T, P, EMB).transpose(1, 0, 2).reshape(P, T * EMB))

        slot = np.full((B_CAP,), U_CAP - 1, np.int64)
        slot[:n] = inv
        slotf = np.ascontiguousarray(slot.reshape(T, P).T.astype(np.float32))

        gidx = np.zeros((U_CAP,), np.int32)
        gidx[:u] = uniq.astype(np.int32)
        # pad slots scatter into the trash row (index `shard`) of the output
        sidx = np.full((U_CAP,), shard, np.int32)
        sidx[:u] = uniq.astype(np.int32)
        invc = np.zeros((U_CAP,), np.float32)
        invc[:u] = (1.0 / cnt).astype(np.float32)

        per_core.append({
            "centers": np.ascontiguousarray(cen[c * shard:(c + 1) * shard]),
            "embp": embp,
            "slotf": slotf,
            "gidx": np.ascontiguousarray(gidx.reshape(G, P).T),
            "sidx": np.ascontiguousarray(sidx.reshape(G, P).T),
            "invc": np.ascontiguousarray(invc.reshape(G, P).T),
        })
        for t in range(T):
            glo = int(slot[t * P]) // P
            ghi = int(slot[t * P + P - 1]) // P
            for g in range(glo, ghi + 1):
                pairs.add((t, g))

    tiles_of = []
    for g in range(G):
        tl = sorted(t for (t, gg) in pairs if gg == g)
        tiles_of.append(tuple(tl) if tl else (0,))

    # For each slot group g, the table-row band it scatters into (union over
    # cores).  Slots are sorted unique labels, so group g covers a contiguous
    # row range per core; the union lets the shared program order scatter g
    # after only the copy chunks overlapping that band.
    bands = []
    for g in range(G):
        lo, hi = shard, -1
        for c in range(N_CORES):
            uniq = infos[c][1]
            u = len(uniq)
            s0, s1 = g * P, min((g + 1) * P, u) - 1
            if s0 >= u:
                continue
            lo = min(lo, int(uniq[s0]))
            hi = max(hi, int(uniq[s1]))
        bands.append((lo, hi))  # empty group: (shard, -1)
    return per_core, shard, B, T, G, tuple(tiles_of), tuple(bands)


FEATURES_ALL = frozenset({"copy", "rows", "groups", "scatter"})
COPY_NCH = 16
COPY_MODE = "d2d"  # "d2d" | "bounce"
SCATTER_INTERLEAVE = True  # issue scatter g right after group g's finalize
PSUM_BUFS = 2
COPY_3WAY = False  # spread copy chunks over sync/scalar/gpsimd queues


def build_program(shard, T, G, tiles_of, bands=None, loop_reps=1,
                  timing_mode=False, features=FEATURES_ALL):
    """Build + compile the per-core Bass program (shared by all 8 cores).

    timing_mode=True turns out_centers into internal DRAM scratch (same
    memory traffic, tiny external outputs) so repeated dispatches are cheap.
    """
    nc = bacc.Bacc("TRN2", target_bir_lowering=False, debug=False,
                   num_devices=N_CORES)
    AF = mybir.ActivationFunctionType
    OP = mybir.AluOpType
    AX = mybir.AxisListType

    centers = nc.dram_tensor("centers", [shard, EMB], F32, kind="ExternalInput")
    embp = nc.dram_tensor("embp", [P, T * EMB], F32, kind="ExternalInput")
    slotf = nc.dram_tensor("slotf", [P, T], F32, kind="ExternalInput")
    gidx = nc.dram_tensor("gidx", [P, G], I32, kind="ExternalInput")
    sidx = nc.dram_tensor("sidx", [P, G], I32, kind="ExternalInput")
    invc = nc.dram_tensor("invc", [P, G], F32, kind="ExternalInput")
    # one extra trash row at index `shard` catches pad-slot scatters
    if timing_mode:
        out_c = nc.dram_tensor("out_centers_scratch", [shard + 1, EMB], F32)
        # tiny observable output reading back one row per copy chunk, so the
        # table writes cannot be elided / left un-awaited in the timing build
        out_probe = nc.dram_tensor("probe", [P, EMB], F32,
                                   kind="ExternalOutput")
    else:
        out_c = nc.dram_tensor("out_centers", [shard + 1, EMB], F32,
                               kind="ExternalOutput")
        out_probe = None
    out_l = nc.dram_tensor("loss_part", [P, 1], F32, kind="ExternalOutput")

    with tile.TileContext(nc) as tc:
        with (
            tc.tile_pool(name="const", bufs=1) as constp,
            tc.tile_pool(name="big", bufs=1) as bigp,
            tc.tile_pool(name="work", bufs=3) as workp,
            tc.tile_pool(name="grp", bufs=3) as grpp,
            tc.tile_pool(name="ps", bufs=PSUM_BUFS, space="PSUM") as psp,
        ):
            iota_i = constp.tile([P, P], I32)
            nc.gpsimd.iota(iota_i[:], pattern=[[1, P]], base=0,
                           channel_multiplier=0)
            iota_f = constp.tile([P, P], F32)
            nc.vector.tensor_copy(iota_f[:], iota_i[:])
            slotf_t = constp.tile([P, T], F32)
            nc.sync.dma_start(out=slotf_t[:], in_=slotf.ap())
            gidx_t = constp.tile([P, G], I32)
            nc.sync.dma_start(out=gidx_t[:], in_=gidx.ap())
            sidx_t = constp.tile([P, G], I32)
            nc.sync.dma_start(out=sidx_t[:], in_=sidx.ap())
            invc_t = constp.tile([P, G], F32)
            nc.sync.dma_start(out=invc_t[:], in_=invc.ap())

            def body(_iv=None):
                # embeddings load first: it heads the sync HWDGE FIFO and
                # unblocks all compute while the bulk copy streams
                X = None
                if "rows" in features:
                    X = bigp.tile([P, T * EMB], F32, tag="X")
                    nc.sync.dma_start(out=X[:], in_=embp.ap())

                # 1) bulk copy of the centers shard into the output table,
                # split across both HWDGE rings (sync + scalar).  Chunk k
                # covers rows [row_lo_k, row_hi_k) so scatters can order
                # against only the chunks they overlap.
                copy_raws = []       # (row_lo, row_hi, raw_inst)
                if "copy" in features and COPY_MODE == "phased":
                    # read-burst / write-burst alternation: superchunk s+1's
                    # read waits superchunk s's write so HBM sees mostly
                    # one-directional bursts instead of mixed r/w traffic
                    flat_i = centers.ap().rearrange("a b -> (a b)")
                    flat_o = out_c.ap().rearrange("a b -> (a b)")
                    nsc = 8
                    rows_per = _ceil_div(shard, nsc)
                    prev_write = None
                    for s in range(nsc):
                        rlo = s * rows_per
                        rhi = min((s + 1) * rows_per, shard)
                        if rlo >= rhi:
                            continue
                        w = (rhi - rlo) * EMB // P
                        cpt = workp.tile([P, w], F32, tag="cpt")
                        nc.sync.dma_start(out=cpt[:],
                                          in_=flat_i[rlo * EMB:rhi * EMB])
                        rd = nc.cur_bb.bb.instructions[-1]
                        if prev_write is not None:
                            tile.add_dep_helper(
                                rd, prev_write,
                                reason="phase-separate HBM read/write bursts")
                        nc.scalar.dma_start(out=flat_o[rlo * EMB:rhi * EMB],
                                            in_=cpt[:])
                        prev_write = nc.cur_bb.bb.instructions[-1]
                        copy_raws.append((rlo, rhi, prev_write))
                elif "copy" in features:
                    flat_i = centers.ap().rearrange("a b -> (a b)")
                    flat_o = out_c.ap().rearrange("a b -> (a b)")
                    nch = COPY_NCH
                    rows_per = _ceil_div(shard, nch)
                    for k in range(nch):
                        rlo = k * rows_per
                        rhi = min((k + 1) * rows_per, shard)
                        if rlo >= rhi:
                            continue
                        if COPY_3WAY:
                            eng = (nc.sync, nc.scalar, nc.gpsimd)[k % 3]
                        else:
                            eng = nc.sync if k % 2 == 0 else nc.scalar
                        if COPY_MODE == "bounce":
                            w = (rhi - rlo) * EMB // P
                            cpt = workp.tile([P, w], F32, tag="cpt")
                            eng.dma_start(out=cpt[:],
                                          in_=flat_i[rlo * EMB:rhi * EMB])
                            eng.dma_start(out=flat_o[rlo * EMB:rhi * EMB],
                                          in_=cpt[:])
                        else:
                            eng.dma_start(out=flat_o[rlo * EMB:rhi * EMB],
                                          in_=flat_i[rlo * EMB:rhi * EMB])
                        copy_raws.append((rlo, rhi, nc.cur_bb.bb.instructions[-1]))

                def finish(lacc_tile):
                    nc.sync.dma_start(out=out_l.ap(), in_=lacc_tile[:])
                    if timing_mode and out_probe is not None and copy_raws:
                        # one strided read touching every copy chunk's range
                        n = len(copy_raws)
                        stride = max(1, shard // P)
                        prb = workp.tile([P, EMB], F32, tag="prb")
                        nc.sync.dma_start(
                            out=prb[:],
                            in_=out_c.ap()[:stride * P:stride, :])
                        nc.sync.dma_start(out=out_probe.ap(), in_=prb[:])

                lacc = workp.tile([P, 1], F32, tag="lacc")
                if "rows" not in features:
                    nc.vector.memset(lacc[:], 0.0)
                    finish(lacc)
                    return
                Xc = bigp.tile([P, T * 2 * EMB], F32, tag="Xc")
                ss_all = bigp.tile([P, T], F32, tag="ss")
                nrm_all = bigp.tile([P, T], F32, tag="nrm")
                inv_all = bigp.tile([P, T], F32, tag="inv")
                for t in range(T):
                    x = X[:, t * EMB:(t + 1) * EMB]
                    sq = workp.tile([P, EMB], F32, tag="sq")
                    nc.scalar.activation(out=sq[:], in_=x, func=AF.Square,
                                         accum_out=ss_all[:, t:t + 1])
                    nc.scalar.sqrt(nrm_all[:, t:t + 1], ss_all[:, t:t + 1])
                    gc = workp.tile([P, 1], F32, tag="gc")
                    nc.vector.tensor_scalar_max(gc[:], nrm_all[:, t:t + 1], 1e-30)
                    nc.vector.reciprocal(inv_all[:, t:t + 1], gc[:])
                    nc.scalar.activation(out=Xc[:, (2 * t) * EMB:(2 * t + 1) * EMB],
                                         in_=x, func=AF.Copy,
                                         scale=inv_all[:, t:t + 1])
                    nc.scalar.activation(out=Xc[:, (2 * t + 1) * EMB:(2 * t + 2) * EMB],
                                         in_=x, func=AF.Copy,
                                         scale=nrm_all[:, t:t + 1])

                nc.vector.tensor_reduce(out=lacc[:], in_=ss_all[:], axis=AX.X,
                                        op=OP.add)

                if "groups" not in features:
                    finish(lacc)
                    return

                # 3) per slot-group: segment sums -> EMA update.  Updated
                # rows land in a resident buffer; all scatters are issued
                # afterwards so a scatter stalled on the bulk copy never
                # head-of-line-blocks the gathers on the Pool engine.
                orow_all = bigp.tile([P, G * EMB], F32, tag="orow_all")
                scatter_state = [None]

                def do_scatter(g):
                    nc.gpsimd.indirect_dma_start(
                        out=out_c.ap(),
                        out_offset=bass.IndirectOffsetOnAxis(
                            ap=sidx_t[:, g:g + 1], axis=0),
                        in_=orow_all[:, g * EMB:(g + 1) * EMB],
                        in_offset=None)
                    raw = nc.cur_bb.bb.instructions[-1]
                    # scatters write disjoint slot rows: drop the conservative
                    # scatter->scatter WAW chain, but keep the semantic
                    # scatter-after-bulk-copy ordering for the copy chunks
                    # this group's row band overlaps
                    if scatter_state[0] is not None:
                        raw.try_remove_dependency(scatter_state[0].name)
                    if bands is not None:
                        blo, bhi = bands[g]
                    else:
                        blo, bhi = 0, shard - 1
                    for rlo, rhi, ci in copy_raws:
                        if rhi > blo and rlo <= bhi:
                            tile.add_dep_helper(
                                raw, ci, reason="scatter lands after bulk copy")
                    scatter_state[0] = raw

                for g in range(G):
                    tl = tiles_of[g]
                    psA = psp.tile([P, 2 * EMB], F32, tag="psA")
                    psQ = psp.tile([P, 1], F32, tag="psQ")
                    for j, t in enumerate(tl):
                        oh = workp.tile([P, P], F32, tag="oh")
                        # oh[i, m] = 1.0 iff slot(i) == g*128 + m
                        nc.vector.tensor_scalar(
                            out=oh[:], in0=iota_f[:],
                            scalar1=slotf_t[:, t:t + 1], scalar2=float(-g * P),
                            op0=OP.subtract, op1=OP.is_equal)
                        st = (j == 0)
                        sp = (j == len(tl) - 1)
                        nc.tensor.matmul(out=psA[:, :], lhsT=oh[:],
                                         rhs=Xc[:, (2 * t) * EMB:(2 * t + 2) * EMB],
                                         start=st, stop=sp)
                        nc.tensor.matmul(out=psQ[:, :], lhsT=oh[:],
                                         rhs=ss_all[:, t:t + 1],
                                         start=st, stop=sp)

                    cg = grpp.tile([P, EMB], F32, tag="cg")
                    nc.gpsimd.indirect_dma_start(
                        out=cg[:], out_offset=None, in_=centers.ap(),
                        in_offset=bass.IndirectOffsetOnAxis(
                            ap=gidx_t[:, g:g + 1], axis=0))
                    mean = grpp.tile([P, EMB], F32, tag="mean")
                    nc.vector.tensor_scalar(out=mean[:], in0=psA[:, 0:EMB],
                                            scalar1=invc_t[:, g:g + 1],
                                            scalar2=None, op0=OP.mult)
                    upd = grpp.tile([P, EMB], F32, tag="upd")
                    nc.vector.tensor_tensor(out=upd[:], in0=cg[:], in1=mean[:],
                                            op=OP.add)
                    squ = grpp.tile([P, EMB], F32, tag="squ")
                    ssu = grpp.tile([P, 1], F32, tag="ssu")
                    nc.scalar.activation(out=squ[:], in_=upd[:], func=AF.Square,
                                         accum_out=ssu[:])
                    nu = grpp.tile([P, 1], F32, tag="nu")
                    # ||0.5*(c+mean)|| = sqrt(0.25 * sum((c+mean)^2))
                    nc.scalar.activation(out=nu[:], in_=ssu[:], func=AF.Sqrt,
                                         scale=0.25)
                    # 2*max(nu, 1e-12); reciprocal gives 0.5/max(nu, 1e-12)
                    nc.vector.tensor_scalar(out=nu[:], in0=nu[:], scalar1=1e-12,
                                            scalar2=2.0, op0=OP.max, op1=OP.mult)
                    rcol = grpp.tile([P, 1], F32, tag="rcol")
                    nc.vector.reciprocal(rcol[:], nu[:])
                    nc.scalar.activation(out=orow_all[:, g * EMB:(g + 1) * EMB],
                                         in_=upd[:], func=AF.Copy,
                                         scale=rcol[:])

                    # loss terms: -2 * c.S2 and q * ||c||^2
                    l1o = grpp.tile([P, EMB], F32, tag="l1o")
                    t1 = grpp.tile([P, 1], F32, tag="t1")
                    nc.vector.tensor_tensor(out=l1o[:], in0=cg[:],
                                            in1=psA[:, EMB:2 * EMB], op=OP.mult)
                    nc.vector.tensor_reduce(out=t1[:], in_=l1o[:], axis=AX.X,
                                            op=OP.add)
                    nc.vector.tensor_scalar(out=t1[:], in0=t1[:], scalar1=-2.0,
                                            scalar2=None, op0=OP.mult)
                    l2o = grpp.tile([P, EMB], F32, tag="l2o")
                    t2 = grpp.tile([P, 1], F32, tag="t2")
                    nc.scalar.activation(out=l2o[:], in_=cg[:], func=AF.Square,
                                         accum_out=t2[:])
                    nc.vector.tensor_tensor(out=t2[:], in0=t2[:],
                                            in1=psQ[:, 0:1], op=OP.mult)
                    nc.vector.tensor_tensor(out=lacc[:], in0=lacc[:], in1=t1[:],
                                            op=OP.add)
                    nc.vector.tensor_tensor(out=lacc[:], in0=lacc[:], in1=t2[:],
                                            op=OP.add)
                    if "scatter" in features and SCATTER_INTERLEAVE:
                        do_scatter(g)

                # 4) scatter pass: all updated rows into the output table
                if "scatter" in features and not SCATTER_INTERLEAVE:
                    for g in range(G):
                        do_scatter(g)

                finish(lacc)

            if loop_reps == 1 and not timing_mode:
                body()
            else:
                with tc.For_i(0, loop_reps, 1) as _i:
                    body(_i)
                    # serialize iterations so the loop delta measures true
                    # single-shot latency, not pipelined throughput
                    tc.strict_bb_all_engine_barrier()

    nc.compile()
    return nc


def kernel(embeddings, centers, labels):
    per_core, shard, B, T, G, tiles_of, bands = route(
        embeddings, centers, labels)
    key = (shard, T, G, tiles_of, bands, 1, False)
    nc = _prog_cache.get(key)
    if nc is None:
        nc = build_program(shard, T, G, tiles_of, bands)
        _prog_cache[key] = nc
    res = run_bass_kernel_spmd(nc, per_core, core_ids=list(range(N_CORES)))
    new_centers = np.concatenate(
        [r["out_centers"][:shard] for r in res.results], axis=0)
    total = sum(float(r["loss_part"].astype(np.float64).sum())
                for r in res.results)
    loss = np.float32(total / (B * EMB))
    return loss, new_centers
